# revision 11
# baseline (speedup 1.0000x reference)
"""Trainium2 Bass kernel for nn_Concentration_61229053772314.

kernel(**inputs) takes the FULL inputs (B=64), shards the batch dim across
8 NeuronCores (pure data parallel, weights replicated), runs a Bass/Tile
kernel via run_bass_kernel_spmd, and reassembles the full outputs.

v2: f32 end-to-end (no gpsimd bf16 cast of ve), compat dots as batched
multiply + segmented tensor_reduce on DVE (no accumulator stalls), 1KB DMA
descriptors via even/odd row pairing, DMA spread over 4 queues, PSUM->SBUF
selection copies on gpsimd.

Row permutation: ve rows are loaded pairwise: partition p holds rows 2p
(v=0) and 2p+1 (v=1). All of compat/softmax/top-k/gather operate in the
permuted index space f: f<128 -> n=2f, f>=128 -> n=2(f-128)+1. The dead
mask is permuted at the SBUF copy; everything else is consistent because
selection one-hots multiply ve tiles laid out in the same permuted space.

Self-contained: only imports the concourse runtime that ships with the
environment; does not read any sibling files.
"""
import math
import os
import sys

for _p in ("/opt/trn_rl_repo", "/root/.axon_site/_ro/trn_rl_repo"):
    if os.path.isdir(_p) and _p not in sys.path:
        sys.path.insert(0, _p)

import numpy as np
import concourse.tile as tile
from concourse import bacc, bass_utils, mybir

F32 = mybir.dt.float32
I32 = mybir.dt.int32
U16 = mybir.dt.uint16
AX = mybir.AxisListType
ALU = mybir.AluOpType
ACTF = mybir.ActivationFunctionType

N_CORES = 8
B, A = 64, 32
N = 256    # entries per (b,a)
H = 128    # head dim
K16 = 16   # top-k
GRP = 64   # (b,a) pairs per processing group
QB = 8     # ba per DMA chunk / compat batch
CH = 16    # ba per xsel PSUM tile

NEG_MASK = -1.0e30   # added to masked entries
NEG_REPL = -3.0e38   # match_replace fill (below any real/masked value)

_CACHE = {}


def _build(nc, B_pc):
    NBA = 32 * B_pc
    assert NBA % GRP == 0
    NG = NBA // GRP

    ve_d = nc.dram_tensor("ve", [NBA, N, H], F32, kind="ExternalInput")
    vs_d = nc.dram_tensor("vs", [NBA, H], F32, kind="ExternalInput")
    dead_d = nc.dram_tensor("dead", [NBA, N], I32, kind="ExternalInput")
    wq_d = nc.dram_tensor("wq", [H, H], F32, kind="ExternalInput")
    wk_d = nc.dram_tensor("wk", [H, H], F32, kind="ExternalInput")
    wv_d = nc.dram_tensor("wv", [H, H], F32, kind="ExternalInput")
    wmot_d = nc.dram_tensor("wmot", [H, 2 * H], F32, kind="ExternalInput")
    bmot_d = nc.dram_tensor("bmot", [H, 1], F32, kind="ExternalInput")
    wfwd_d = nc.dram_tensor("wfwd", [H, (K16 + 1) * H], F32, kind="ExternalInput")
    bfwd_d = nc.dram_tensor("bfwd", [H, 1], F32, kind="ExternalInput")
    vc_d = nc.dram_tensor("vc", [NBA, H], F32, kind="ExternalOutput")
    vm_d = nc.dram_tensor("vm", [NBA, H], F32, kind="ExternalOutput")

    with tile.TileContext(nc) as tc:
        _body(nc, tc, NBA, NG, ve_d, vs_d, dead_d, wq_d, wk_d, wv_d,
              wmot_d, bmot_d, wfwd_d, bfwd_d, vc_d, vm_d)


def _body(nc, tc, NBA, NG, ve_d, vs_d, dead_d, wq_d, wk_d, wv_d,
          wmot_d, bmot_d, wfwd_d, bfwd_d, vc_d, vm_d):
    from contextlib import ExitStack
    NCH = GRP // QB          # compat chunks per group
    NT16 = GRP // 16         # t-broadcast tiles per group
    with ExitStack() as ctx:
        consts = ctx.enter_context(tc.tile_pool(name="consts", bufs=1))
        wpool = ctx.enter_context(tc.tile_pool(name="weights", bufs=1))
        grp_pool = ctx.enter_context(tc.tile_pool(name="grp", bufs=2))
        vst_pool = ctx.enter_context(tc.tile_pool(name="vst", bufs=NG))
        ve_pool = ctx.enter_context(tc.tile_pool(name="venat", bufs=10))
        tb_pool = ctx.enter_context(tc.tile_pool(name="tb", bufs=3))
        scr_pool = ctx.enter_context(tc.tile_pool(name="scr", bufs=2))
        small = ctx.enter_context(tc.tile_pool(name="small", bufs=3))
        dram_pool = ctx.enter_context(tc.tile_pool(name="dram", bufs=NG, space="DRAM"))
        ps_xsel = ctx.enter_context(tc.tile_pool(name="ps_xsel", bufs=2, space="PSUM"))
        ps_tr = ctx.enter_context(tc.tile_pool(name="ps_tr", bufs=2, space="PSUM"))

        # constants: iotas -> identity
        iota_n = consts.tile([128, N], I32)
        nc.gpsimd.iota(iota_n[:], pattern=[[1, N]], base=0, channel_multiplier=0)
        iota_p = consts.tile([128, 1], F32)
        nc.gpsimd.iota(iota_p[:], pattern=[[0, 1]], base=0, channel_multiplier=1,
                       allow_small_or_imprecise_dtypes=True)
        ident_f = consts.tile([128, 128], F32)
        nc.vector.tensor_scalar(ident_f[:], iota_n[:, 0:128], iota_p[:], None,
                                op0=ALU.is_equal)

        # weights
        wq = wpool.tile([H, H], F32)
        nc.sync.dma_start(wq[:], wq_d.ap())
        wk = wpool.tile([H, H], F32)
        nc.sync.dma_start(wk[:], wk_d.ap())
        wv = wpool.tile([H, H], F32)
        nc.sync.dma_start(wv[:], wv_d.ap())
        wmot = wpool.tile([H, 2 * H], F32)
        nc.sync.dma_start(wmot[:], wmot_d.ap())
        wfwd = wpool.tile([H, (K16 + 1) * H], F32)
        nc.sync.dma_start(wfwd[:], wfwd_d.ap())
        bmot = wpool.tile([H, 1], F32)
        nc.sync.dma_start(bmot[:], bmot_d.ap())
        bfwd = wpool.tile([H, 1], F32)
        nc.sync.dma_start(bfwd[:], bfwd_d.ap())

        def pe_transpose_f32(dst_sb, src_sb, n_in=128, n_out=128):
            ps = ps_tr.tile([128, 256], F32, tag="tr")
            nc.tensor.transpose(ps[0:n_out, 0:n_in], src_sb, ident_f[0:n_in, 0:n_in])
            nc.scalar.copy(dst_sb, ps[0:n_out, 0:n_in])

        wkT = wpool.tile([H, H], F32)
        pe_transpose_f32(wkT[:], wk[:])
        wvT = wpool.tile([H, H], F32)
        pe_transpose_f32(wvT[:], wv[:])
        wm0T = wpool.tile([H, H], F32)
        pe_transpose_f32(wm0T[:], wmot[:, 0:H])
        wm1T = wpool.tile([H, H], F32)
        pe_transpose_f32(wm1T[:], wmot[:, H:2 * H])

        # WmvT[iu,o] = sum_i2 WvT[i2,iu] * Wm1T[i2,o]  (= (Wm1 @ Wv^T)^T)
        wmvT_f = wpool.tile([H, H], F32)
        ps = ps_tr.tile([128, 256], F32, tag="tr")
        nc.tensor.matmul(ps[:, 0:128], wvT[:], wm1T[:])
        nc.scalar.copy(wmvT_f[:], ps[:, 0:128])

        # W_fwd block transposes -> [h, ho] blocks packed [128, 17*128]
        wfT = wpool.tile([H, (K16 + 1) * H], F32)
        for j in range(K16 + 1):
            pe_transpose_f32(wfT[:, j * H:(j + 1) * H], wfwd[:, j * H:(j + 1) * H])

        # per-group precompute: VST (vs transposed), T = (Wk @ Wq^T vs)/sqrt(H)
        vst_f, t_dr = [], []
        for g in range(NG):
            vs_rows = small.tile([GRP, H], F32, tag="vsrows")
            nc.gpsimd.dma_start(vs_rows[:], vs_d.ap()[g * GRP:(g + 1) * GRP, :])
            vstf = vst_pool.tile([H, GRP], F32, tag="vstf")
            pe_transpose_f32(vstf[:], vs_rows[:], n_in=GRP)
            qt = grp_pool.tile([H, GRP], F32, tag="qt")
            ps = ps_tr.tile([128, 256], F32, tag="tr")
            nc.tensor.matmul(ps[:, 0:GRP], wq[:], vstf[:])
            nc.scalar.copy(qt[:], ps[:, 0:GRP])
            tsb = grp_pool.tile([H, GRP], F32, tag="tsb")
            ps = ps_tr.tile([128, 256], F32, tag="tr")
            nc.tensor.matmul(ps[:, 0:GRP], wkT[:], qt[:])
            nc.scalar.mul(tsb[:], ps[:, 0:GRP], 1.0 / math.sqrt(H))
            # t rows to DRAM so they can be partition-broadcast later
            trows = grp_pool.tile([GRP, H], F32, tag="trows")
            pe_transpose_f32(trows[:], tsb[:], n_in=128, n_out=GRP)
            t_dram = dram_pool.tile([GRP, H], F32, tag="tdram")
            nc.gpsimd.dma_start(t_dram[:], trows[:])
            vst_f.append(vstf)
            t_dr.append(t_dram)

        dma_engs = [nc.sync, nc.scalar]

        for g in range(NG):
            # ---- stage 1: load ve chunks, compat dots (mult + seg-reduce) ----
            # cc[p, v*GRP + b] = compat of row (2p+v) of ba b   (f32)
            cc = grp_pool.tile([128, 2 * GRP], F32, tag="cc")
            cc_w = cc[:].rearrange("p (v b) -> p b v", v=2)
            venat_g = []
            tb16 = None
            for c in range(NCH):
                ib = g * GRP + c * QB
                if c % 2 == 0:
                    tb16 = tb_pool.tile([128, 16 * H], F32, tag="t16")
                    nc.gpsimd.dma_start(
                        tb16[:], t_dr[g][:][c * QB:c * QB + 16, :]
                        .rearrange("b h -> (b h)").partition_broadcast(128))
                tb_v = tb16[:].rearrange("p (b h) -> p b h", b=16)
                boff = (c % 2) * QB

                venat = ve_pool.tile([128, QB * N], F32, tag="venat")
                vv = venat[:].rearrange("p (b v h) -> p b v h", b=QB, v=2)
                src = ve_d.ap()[ib:ib + QB].rearrange(
                    "b (p v) h -> p b v h", v=2)
                dma_engs[c % 2].dma_start(vv, src)

                scr = scr_pool.tile([128, QB * N], F32, tag="scr")
                sv = scr[:].rearrange("p (b v h) -> p b v h", b=QB, v=2)
                for v in range(2):
                    nc.vector.scalar_tensor_tensor(
                        sv[:, :, v, :], vv[:, :, v, :], 1.0,
                        tb_v[:, boff:boff + QB, :],
                        op0=ALU.mult, op1=ALU.mult)
                nc.vector.tensor_reduce(
                    cc_w[:, c * QB:(c + 1) * QB, :],
                    scr[:].rearrange("p (s h) -> p s h", h=H),
                    axis=AX.X, op=ALU.add)
                venat_g.append(venat)

            # ---- stage 2: mask, softmax, top-16, one-hot/score transposes ----
            cmp_ps = ps_tr.tile([128, 256], F32, tag="st")
            nc.tensor.transpose(cmp_ps[0:GRP, 0:128], cc[:, 0:GRP],
                                ident_f[:])
            nc.tensor.transpose(cmp_ps[0:GRP, 128:256], cc[:, GRP:2 * GRP],
                                ident_f[:])

            dead_i = grp_pool.tile([GRP, N], I32, tag="deadi")
            nc.gpsimd.dma_start(dead_i[:], dead_d.ap()[g * GRP:(g + 1) * GRP, :])
            dead_f = grp_pool.tile([GRP, N], F32, tag="deadf")
            nc.gpsimd.tensor_copy(
                dead_f[:].rearrange("p (v x) -> p v x", v=2),
                dead_i[:].rearrange("p (x v) -> p v x", v=2))
            cm_sb = grp_pool.tile([GRP, N], F32, tag="cmsb")
            nc.vector.scalar_tensor_tensor(cm_sb[:], dead_f[:], NEG_MASK,
                                           cmp_ps[0:GRP, :],
                                           op0=ALU.mult, op1=ALU.add)

            mx_neg = small.tile([GRP, 1], F32, tag="mxneg")
            nc.vector.tensor_reduce(mx_neg[:], cm_sb[:], axis=AX.X, op=ALU.max,
                                    negate=True)
            score_un = grp_pool.tile([GRP, N], F32, tag="scoreun")
            ssum = small.tile([GRP, 1], F32, tag="ssum")
            nc.scalar.activation(score_un[:], cm_sb[:], ACTF.Exp,
                                 bias=mx_neg[:], scale=1.0, accum_out=ssum[:])
            rs = small.tile([GRP, 1], F32, tag="rs")
            nc.vector.reciprocal(rs[:], ssum[:])
            score_f = grp_pool.tile([GRP, N], F32, tag="scoref")
            nc.vector.tensor_scalar_mul(score_f[:], score_un[:], rs[:])

            mx8a = small.tile([GRP, 8], F32, tag="mx8a")
            nc.vector.max(mx8a[:], cm_sb[:])
            idx16 = small.tile([GRP, K16], U16, tag="idx16")
            nc.vector.max_index(idx16[:, 0:8], mx8a[:], cm_sb[:])
            cm2 = grp_pool.tile([GRP, N], F32, tag="cm2")
            nc.vector.match_replace(cm2[:], mx8a[:], cm_sb[:], NEG_REPL)
            mx8b = small.tile([GRP, 8], F32, tag="mx8b")
            nc.vector.max(mx8b[:], cm2[:])
            nc.vector.max_index(idx16[:, 8:16], mx8b[:], cm2[:])
            idx_f = small.tile([GRP, K16], F32, tag="idxf")
            nc.gpsimd.tensor_copy(idx_f[:], idx16[:])

            # s_a: selection weights for even rows (f<128), s_b: odd rows
            s_a = grp_pool.tile([128, GRP * (K16 + 1)], F32, tag="sa")
            s_b = grp_pool.tile([128, GRP * (K16 + 1)], F32, tag="sb")
            s_a_v = s_a[:].rearrange("p (b j) -> p b j", j=K16 + 1)
            s_b_v = s_b[:].rearrange("p (b j) -> p b j", j=K16 + 1)
            for j in range(K16):
                sr = grp_pool.tile([GRP, N], F32, tag="srj")
                nc.vector.tensor_scalar(sr[:], iota_n[0:GRP, :],
                                        idx_f[:, j:j + 1], None,
                                        op0=ALU.is_equal)
                st_ps = ps_tr.tile([128, 256], F32, tag="st")
                nc.tensor.transpose(st_ps[:, 0:GRP], sr[:, 0:128],
                                    ident_f[0:GRP, 0:GRP])
                nc.tensor.transpose(st_ps[:, GRP:2 * GRP], sr[:, 128:256],
                                    ident_f[0:GRP, 0:GRP])
                nc.scalar.copy(s_a_v[:, :, j], st_ps[:, 0:GRP])
                nc.scalar.copy(s_b_v[:, :, j], st_ps[:, GRP:2 * GRP])
            st_ps = ps_tr.tile([128, 256], F32, tag="st")
            nc.tensor.transpose(st_ps[:, 0:GRP], score_f[:, 0:128],
                                ident_f[0:GRP, 0:GRP])
            nc.tensor.transpose(st_ps[:, GRP:2 * GRP], score_f[:, 128:256],
                                ident_f[0:GRP, 0:GRP])
            nc.scalar.copy(s_a_v[:, :, K16], st_ps[:, 0:GRP])
            nc.scalar.copy(s_b_v[:, :, K16], st_ps[:, GRP:2 * GRP])

            # ---- stage 3: per-ba [gathered rows | u] = ve^T @ [one-hot|score] ----
            xq_sb = grp_pool.tile([128, GRP * (K16 + 1)], F32, tag="xq")
            u_f = grp_pool.tile([128, GRP], F32, tag="uf")
            xsel_ps = None
            for col in range(GRP):
                pos = col % CH
                if pos == 0:
                    xsel_ps = ps_xsel.tile([128, CH * (K16 + 1)], F32, tag="xsel")
                vv = venat_g[col // QB][:].rearrange(
                    "p (b v h) -> p b v h", b=QB, v=2)
                lo, hi = pos * 17, pos * 17 + 17
                nc.tensor.matmul(xsel_ps[:, lo:hi], vv[:, col % QB, 0, :],
                                 s_a[:, col * 17:(col + 1) * 17],
                                 start=True, stop=False)
                nc.tensor.matmul(xsel_ps[:, lo:hi], vv[:, col % QB, 1, :],
                                 s_b[:, col * 17:(col + 1) * 17],
                                 start=False, stop=True)
                if pos == CH - 1:
                    c0 = (col // CH) * CH
                    nc.scalar.copy(xq_sb[:, c0 * 17:(col + 1) * 17],
                                   xsel_ps[:, 0:CH * 17])
                    xv = xsel_ps[:].rearrange("p (b j) -> p b j", j=K16 + 1)
                    nc.scalar.copy(u_f[:, c0:col + 1], xv[:, :, K16])

            xq_v = xq_sb[:].rearrange("p (b j) -> p b j", j=K16 + 1)

            # vC = relu(Wfwd @ [vs | gathered] + bfwd)
            vc_ps = ps_tr.tile([128, 256], F32, tag="tr")
            nc.tensor.matmul(vc_ps[:, 0:GRP], wfT[:, 0:H], vst_f[g][:],
                             start=True, stop=False)
            for j in range(1, K16 + 1):
                nc.tensor.matmul(vc_ps[:, 0:GRP], wfT[:, j * H:(j + 1) * H],
                                 xq_v[:, :, j - 1],
                                 start=False, stop=(j == K16))
            vc_sb = grp_pool.tile([128, GRP], F32, tag="vcsb")
            nc.scalar.activation(vc_sb[:], vc_ps[:, 0:GRP], ACTF.Relu,
                                 bias=bfwd[:], scale=1.0)
            vc_rows = grp_pool.tile([GRP, H], F32, tag="vcrows")
            pe_transpose_f32(vc_rows[:], vc_sb[:], n_in=128, n_out=GRP)
            nc.gpsimd.dma_start(vc_d.ap()[g * GRP:(g + 1) * GRP, :], vc_rows[:])

            # vM = relu(Wm0 @ vs + Wmv @ u + bmot)
            vm_ps = ps_tr.tile([128, 256], F32, tag="tr")
            nc.tensor.matmul(vm_ps[:, 0:GRP], wm0T[:], vst_f[g][:],
                             start=True, stop=False)
            nc.tensor.matmul(vm_ps[:, 0:GRP], wmvT_f[:], u_f[:],
                             start=False, stop=True)
            vm_sb = grp_pool.tile([128, GRP], F32, tag="vmsb")
            nc.scalar.activation(vm_sb[:], vm_ps[:, 0:GRP], ACTF.Relu,
                                 bias=bmot[:], scale=1.0)
            vm_rows = grp_pool.tile([GRP, H], F32, tag="vmrows")
            pe_transpose_f32(vm_rows[:], vm_sb[:], n_in=128, n_out=GRP)
            nc.gpsimd.dma_start(vm_d.ap()[g * GRP:(g + 1) * GRP, :], vm_rows[:])


def _get_compiled(B_pc):
    key = B_pc
    if key not in _CACHE:
        nc = bacc.Bacc("TRN2", target_bir_lowering=False, debug=False,
                       num_devices=N_CORES)
        _build(nc, B_pc)
        nc.compile()
        _CACHE[key] = nc
    return _CACHE[key]


def kernel(vs, ve, ve_dead, Wq, Wk, Wv, W_mot, b_mot, W_fwd, b_fwd,
           trace=False, trace_kwargs=None):
    vs = np.asarray(vs, dtype=np.float32)
    ve = np.asarray(ve, dtype=np.float32)
    ve_dead = np.asarray(ve_dead, dtype=np.int32)
    Bq, Aq = vs.shape[0], vs.shape[1]
    assert (Bq, Aq) == (B, A), (Bq, Aq)
    B_pc = B // N_CORES
    NBA = B_pc * A

    nc = _get_compiled(B_pc)

    shared = {
        "wq": np.ascontiguousarray(Wq, dtype=np.float32),
        "wk": np.ascontiguousarray(Wk, dtype=np.float32),
        "wv": np.ascontiguousarray(Wv, dtype=np.float32),
        "wmot": np.ascontiguousarray(W_mot, dtype=np.float32),
        "bmot": np.ascontiguousarray(b_mot, dtype=np.float32).reshape(H, 1),
        "wfwd": np.ascontiguousarray(W_fwd, dtype=np.float32),
        "bfwd": np.ascontiguousarray(b_fwd, dtype=np.float32).reshape(H, 1),
    }
    in_maps = []
    for c in range(N_CORES):
        sl = slice(c * B_pc, (c + 1) * B_pc)
        in_maps.append({
            "ve": np.ascontiguousarray(ve[sl].reshape(NBA, N, H)),
            "vs": np.ascontiguousarray(vs[sl].reshape(NBA, H)),
            "dead": np.ascontiguousarray(ve_dead[sl].reshape(NBA, N)),
            **shared,
        })

    res = bass_utils.run_bass_kernel_spmd(
        nc, in_maps, core_ids=list(range(N_CORES)),
        trace=trace, **(trace_kwargs or {}))

    vc = np.empty((B, A, H), dtype=np.float32)
    vm = np.empty((B, A, H), dtype=np.float32)
    for c in range(N_CORES):
        sl = slice(c * B_pc, (c + 1) * B_pc)
        vc[sl] = res.results[c]["vc"].reshape(B_pc, A, H)
        vm[sl] = res.results[c]["vm"].reshape(B_pc, A, H)
    kernel.last_results = res
    return (vc, vm)
